# revision 1
# baseline (speedup 1.0000x reference)
"""Trainium2 Bass kernel for nn_MultiHeadDynamics.

Computation (per sample row x of state, s of signal):
    heads   = x.reshape(H, DH)                      # H=16, DH=256
    A_h     = U_h @ V_h + diag(d_h)                 # (DH, DH) per head
    lin     = heads @ A_h^T
    c       = heads - mean_dh(heads)
    drift   = lin + cs * c^3 + s
    out     = x + DT*(1+cp)*drift - (DT*cp/H) * sum_h(drift_h)

Folding:  beta = DT*(1+cp);  gp = DT*cp/(H*beta);  sq = sqrt(beta*cs)
    D'      = beta*drift = heads @ (beta*A)^T + Square(sq*c)*c + beta*s
    out     = x + D' - gp * sum_h(D'_h)

Sharding: batch B=8192 split across 8 cores (1024 rows each), params
replicated. Per core, rows are processed as 8 tiles of [128, 4096].
The head contraction needs d on partitions, so each [128,128] chunk of
the state tile is PE-transposed; transposed chunks serve as matmul
stationary operands against precomputed (beta*A)^T, with a fused
ones-vector matmul producing the within-head means for free.
"""

import sys

for _p in ("/opt/trn_rl_repo",):
    if _p not in sys.path:
        sys.path.insert(0, _p)

import math
from contextlib import ExitStack

import numpy as np

import concourse.bass as bass
import concourse.tile as tile
from concourse import bacc, mybir
from concourse.bass_utils import run_bass_kernel_spmd
from concourse.masks import make_identity

F32 = mybir.dt.float32
AOP = mybir.AluOpType

# Problem constants (full-input shapes; hardcoded per the task contract).
B = 8192
D = 4096
H = 16
DH = 256
R = 64
DT = 0.05
NCORES = 8
BS = B // NCORES          # rows per core = 1024
P = 128                   # partitions
NT = BS // P              # row tiles per core = 8
NCH = D // P              # 128-wide column chunks per row tile = 32

# Matmul dtype: bfloat16 keeps PE fast (1 cyc/row) with ~4e-5 output
# error; float32 is exact but 4 cyc/row.
MM_DTYPE = mybir.dt.bfloat16
BF16 = mybir.dt.bfloat16
# Middle elementwise chain dtype: fp16 has a 10-bit mantissa (8x finer
# than bf16) and still gets the DVE 16-bit 2x packing mode.
MID = mybir.dt.float16

# Columns of the final fp32 (x + dd) pass handled by DVE; the rest on
# GpSimd. fp32 tensor_tensor is 1x on DVE, ~2x worse on GpSimd.
FINAL_DVE_COLS = 1024

# Fold drift = lin + t2 into PSUM via identity matmuls on the PE
# (software-pipelined one tile behind so the PE never waits on t2).
IDENT_MM = True


def _emit(tc: tile.TileContext, aps: dict, cubic_scale: float, coupling: float):
    nc = tc.nc
    beta = DT * (1.0 + coupling)
    gp = DT * coupling / (H * beta)
    sq = math.sqrt(beta * cubic_scale)

    state = aps["state"]
    signal = aps["signal"]
    U_d = aps["U"]
    V_d = aps["V"]
    diag_d = aps["diag"]
    out_d = aps["out"]

    with ExitStack() as ctx:
        consts = ctx.enter_context(tc.tile_pool(name="consts", bufs=1))

        ident = consts.tile([P, P], F32, tag="ident")
        make_identity(nc, ident)
        ident_bf = consts.tile([P, P], BF16, tag="ident_bf")
        make_identity(nc, ident_bf)


        # Diagonal-position masks for the two 128-chunks of a head.
        dmasks = []
        for k in range(2):
            dmask = consts.tile([P, DH], F32, tag=f"dmask{k}")
            nc.gpsimd.memset(dmask, 0.0)
            nc.gpsimd.affine_select(
                out=dmask, in_=dmask,
                compare_op=AOP.not_equal, fill=1.0,
                base=-(k * P), pattern=[[1, DH]], channel_multiplier=-1,
            )
            dmasks.append(dmask)

        ones = consts.tile([P, 1], MM_DTYPE, tag="ones")
        nc.gpsimd.memset(ones, 1.0 / DH)

        # (beta*A)^T, laid out [d-chunk partition, head, chunk, e].
        AT = consts.tile([P, H, 2, DH], MM_DTYPE, tag="AT")

        # --- one-time A setup ---
        with (
            tc.tile_pool(name="setup", bufs=2) as setup,
            tc.tile_pool(name="setup_ps", bufs=2, space="PSUM") as setup_ps,
        ):
            for h in range(H):
                u_s = setup.tile([P, 2, R], F32, tag="u_s")
                nc.sync.dma_start(out=u_s, in_=U_d[h].rearrange("(k p) r -> p k r", p=P))
                v_s = setup.tile([R, DH], F32, tag="v_s")
                nc.sync.dma_start(out=v_s, in_=V_d[h])
                dcol = setup.tile([P, 2], F32, tag="dcol")
                nc.sync.dma_start(
                    out=dcol, in_=diag_d[h].rearrange("(k p) -> p k", p=P)
                )

                # U_h^T via PE transpose: [128,64] chunks -> [64,128]
                ut_s = setup.tile([R, DH], F32, tag="ut_s")
                for k in range(2):
                    ut_ps = setup_ps.tile([R, P], F32, tag="ut_ps")
                    nc.tensor.transpose(ut_ps, u_s[:, k, :], ident)
                    nc.scalar.copy(out=ut_s[:, k * P:(k + 1) * P], in_=ut_ps)

                for k in range(2):
                    # (V^T U^T) chunk: [d=128, e=256]
                    a_ps = setup_ps.tile([P, DH], F32, tag="a_ps")
                    nc.tensor.matmul(
                        a_ps, lhsT=v_s[:, k * P:(k + 1) * P], rhs=ut_s,
                        start=True, stop=True,
                    )
                    # beta * diag embedded on the diagonal of this chunk
                    dg = setup.tile([P, DH], F32, tag="dg")
                    nc.vector.tensor_scalar(
                        out=dg, in0=dmasks[k],
                        scalar1=dcol[:, k:k + 1], scalar2=beta,
                        op0=AOP.mult, op1=AOP.mult,
                    )
                    # AT[:, h, k, :] = beta*(V^T U^T) + beta*diag, cast
                    nc.vector.scalar_tensor_tensor(
                        out=AT[:, h, k, :], in0=a_ps, scalar=beta, in1=dg,
                        op0=AOP.mult, op1=AOP.add,
                    )

        # --- main loop pools ---
        xp = ctx.enter_context(tc.tile_pool(name="xp", bufs=3))
        sp = ctx.enter_context(tc.tile_pool(name="sp", bufs=2))
        tp = ctx.enter_context(tc.tile_pool(name="tp", bufs=1))
        hp = ctx.enter_context(tc.tile_pool(name="hp", bufs=2))
        mp = ctx.enter_context(tc.tile_pool(name="mp", bufs=2))
        trp = ctx.enter_context(tc.tile_pool(name="trp", bufs=2))
        ps_tp = ctx.enter_context(tc.tile_pool(name="ps_tp", bufs=2, space="PSUM"))
        ps_lin = ctx.enter_context(tc.tile_pool(name="ps_lin", bufs=3, space="PSUM"))
        ps_m = ctx.enter_context(tc.tile_pool(name="ps_m", bufs=1, space="PSUM"))

        for it in range(NT):
            r0 = it * P
            # split input streams across the two HWDGE queues (SP / ACT)
            x_t = xp.tile([P, D], F32, tag="x", name="x_t")
            nc.sync.dma_start(out=x_t, in_=state[r0:r0 + P, :])
            s_t = sp.tile([P, D], F32, tag="s", name="s_t")
            nc.scalar.dma_start(out=s_t, in_=signal[r0:r0 + P, :])

            # beta*s in fp16, off the critical chain (DVE 2x)
            sb_t = tp.tile([P, D], MID, tag="sb", name="sb_t")
            nc.vector.tensor_scalar(
                out=sb_t, in0=s_t, scalar1=beta, scalar2=None, op0=AOP.mult,
            )

            # Transpose all 32 f32 chunks of x into hT (d on partitions),
            # casting to bf16 in the PSUM->SBUF copy.
            hT = hp.tile([P, NCH, P], MM_DTYPE, tag="hT", name="hT")
            for g in range(NCH // 8):
                tp_ps = ps_tp.tile([P, 8 * P], F32, tag="tp_ps", name="tp_ps")
                for c8 in range(8):
                    j = g * 8 + c8
                    nc.tensor.transpose(
                        tp_ps[:, c8 * P:(c8 + 1) * P],
                        x_t[:, j * P:(j + 1) * P], ident,
                    )
                nc.scalar.copy(
                    out=hT[:, g * 8:(g + 1) * 8, :].rearrange("p a b -> p (a b)"),
                    in_=tp_ps,
                )

            # Per-head matmuls: lin' per head pair; within-head means via
            # the ones vector (value 1/DH) as an extra cheap matmul.
            m_ps = ps_m.tile([P, H], F32, tag="m_ps", name="m_ps")
            lin_t = tp.tile([P, D], MID, tag="lin", name="lin_t")
            for hp2 in range(H // 2):
                l_ps = ps_lin.tile([P, 2 * DH], F32, tag="l_ps", name="l_ps")
                for hh in range(2):
                    h = hp2 * 2 + hh
                    for k in range(2):
                        j = 2 * h + k
                        nc.tensor.matmul(
                            l_ps[:, hh * DH:(hh + 1) * DH],
                            lhsT=hT[:, j, :], rhs=AT[:, h, k, :],
                            start=(k == 0), stop=(k == 1),
                        )
                        nc.tensor.matmul(
                            m_ps[:, h:h + 1],
                            lhsT=hT[:, j, :], rhs=ones,
                            start=(k == 0), stop=(k == 1),
                        )
                nc.scalar.copy(
                    out=lin_t[:, hp2 * 2 * DH:(hp2 + 1) * 2 * DH], in_=l_ps
                )
            m_t = mp.tile([P, H], F32, tag="m", name="m_t")
            nc.scalar.copy(out=m_t, in_=m_ps)
            msq_t = mp.tile([P, H], F32, tag="msq", name="msq_t")
            nc.scalar.mul(msq_t, m_ps, -sq)

            # c2 = beta*cs*(x-m)^2 straight from x on ACT (bias trick)
            c2_t = tp.tile([P, D], MID, tag="c2", name="c2_t")
            for h in range(H):
                nc.scalar.activation(
                    out=c2_t[:, h * DH:(h + 1) * DH],
                    in_=x_t[:, h * DH:(h + 1) * DH],
                    func=mybir.ActivationFunctionType.Square,
                    scale=sq, bias=msq_t[:, h:h + 1],
                )
            # c3 = (x - m) * c2 fused per segment
            c3_t = tp.tile([P, D], MID, tag="c3", name="c3_t")
            for h in range(H):
                nc.vector.scalar_tensor_tensor(
                    out=c3_t[:, h * DH:(h + 1) * DH],
                    in0=x_t[:, h * DH:(h + 1) * DH],
                    scalar=m_t[:, h:h + 1],
                    in1=c2_t[:, h * DH:(h + 1) * DH],
                    op0=AOP.subtract, op1=AOP.mult,
                )
            # t2 = beta*s + c3 (fp16 2x)
            t2_t = tp.tile([P, D], MID, tag="t2", name="t2_t")
            nc.vector.tensor_add(t2_t, sb_t, c3_t)
            # drift = lin' + t2 (fp16 2x; reuse c3's buffer)
            dr_t = c3_t
            nc.vector.tensor_add(dr_t, lin_t, t2_t)

            # head-sum tree, flat contiguous halves (order-independent sum)
            t8 = trp.tile([P, D // 2], MID, tag="t8", name="t8")
            nc.vector.tensor_add(t8, dr_t[:, 0:D // 2], dr_t[:, D // 2:D])
            t4 = trp.tile([P, D // 4], MID, tag="t4", name="t4")
            nc.vector.tensor_add(t4, t8[:, 0:D // 4], t8[:, D // 4:D // 2])
            t2r = trp.tile([P, D // 8], MID, tag="t2r", name="t2r")
            nc.vector.tensor_add(t2r, t4[:, 0:D // 8], t4[:, D // 8:D // 4])
            # mhn2 = two side-by-side copies of -gp*sum_h(drift)
            mhn2 = trp.tile([P, 2 * DH], MID, tag="mhn2", name="mhn2")
            nc.vector.tensor_add(mhn2[:, 0:DH], t2r[:, 0:DH], t2r[:, DH:2 * DH])
            nc.vector.tensor_scalar_mul(mhn2[:, 0:DH], mhn2[:, 0:DH], -gp)
            nc.vector.tensor_copy(mhn2[:, DH:2 * DH], mhn2[:, 0:DH])

            # dd = drift + mhn (head-pair flat adds; reuse c2's buffer)
            dd_t = c2_t
            for hp2 in range(H // 2):
                nc.vector.tensor_add(
                    dd_t[:, hp2 * 2 * DH:(hp2 + 1) * 2 * DH],
                    dr_t[:, hp2 * 2 * DH:(hp2 + 1) * 2 * DH], mhn2,
                )

            # out = x + dd (fp32+fp16 mixed, split DVE / GpSimd; into x buf)
            o_t = x_t
            ncol = FINAL_DVE_COLS
            if ncol > 0:
                nc.vector.tensor_add(
                    o_t[:, 0:ncol], x_t[:, 0:ncol], dd_t[:, 0:ncol]
                )
            if ncol < D:
                nc.gpsimd.tensor_add(
                    o_t[:, ncol:D], x_t[:, ncol:D], dd_t[:, ncol:D]
                )
            if it % 2 == 0:
                nc.sync.dma_start(out=out_d[r0:r0 + P, :], in_=o_t)
            else:
                nc.scalar.dma_start(out=out_d[r0:r0 + P, :], in_=o_t)



_CACHE: dict = {}


def _build(cubic_scale: float, coupling: float) -> bass.Bass:
    key = (float(cubic_scale), float(coupling), MM_DTYPE, FINAL_DVE_COLS)
    if key in _CACHE:
        return _CACHE[key]
    nc = bacc.Bacc("TRN2", target_bir_lowering=False, debug=False)
    aps = {
        "state": nc.dram_tensor("state", [BS, D], F32, kind="ExternalInput").ap(),
        "signal": nc.dram_tensor("signal", [BS, D], F32, kind="ExternalInput").ap(),
        "U": nc.dram_tensor("U", [H, DH, R], F32, kind="ExternalInput").ap(),
        "V": nc.dram_tensor("V", [H, R, DH], F32, kind="ExternalInput").ap(),
        "diag": nc.dram_tensor("diag", [H, DH], F32, kind="ExternalInput").ap(),
        "out": nc.dram_tensor("out", [BS, D], F32, kind="ExternalOutput").ap(),
    }
    with tile.TileContext(nc) as tc:
        _emit(tc, aps, float(cubic_scale), float(coupling))
    nc.compile()
    _CACHE[key] = nc
    return nc


def run(state, signal, U, V, diag, cubic_scale, coupling, trace=False):
    state = np.ascontiguousarray(np.asarray(state, dtype=np.float32))
    signal = np.ascontiguousarray(np.asarray(signal, dtype=np.float32))
    U = np.ascontiguousarray(np.asarray(U, dtype=np.float32))
    V = np.ascontiguousarray(np.asarray(V, dtype=np.float32))
    diag = np.ascontiguousarray(np.asarray(diag, dtype=np.float32))

    nc = _build(float(cubic_scale), float(coupling))
    in_maps = []
    for i in range(NCORES):
        sl = slice(i * BS, (i + 1) * BS)
        in_maps.append({
            "state": state[sl], "signal": signal[sl],
            "U": U, "V": V, "diag": diag,
        })
    res = run_bass_kernel_spmd(nc, in_maps, list(range(NCORES)), trace=trace)
    out = np.concatenate([res.results[i]["out"] for i in range(NCORES)], axis=0)
    return out, res


def kernel(state, signal, U, V, diag, cubic_scale, coupling) -> np.ndarray:
    out, _ = run(state, signal, U, V, diag, cubic_scale, coupling, trace=False)
    return out



# revision 9
# speedup vs baseline: 1.2828x; 1.2828x over previous
"""Trainium2 Bass kernel for nn_MultiHeadDynamics.

Computation (per sample row x of state, s of signal):
    heads   = x.reshape(H, DH)                      # H=16, DH=256
    A_h     = U_h @ V_h + diag(d_h)                 # (DH, DH) per head
    lin     = heads @ A_h^T
    c       = heads - mean_dh(heads)
    drift   = lin + cs * c^3 + s
    out     = x + DT*(1+cp)*drift - (DT*cp/H) * sum_h(drift_h)

Folding:  beta = DT*(1+cp);  gp = DT*cp/(H*beta);  alpha = cbrt(beta*cs)
    D'  = beta*drift = heads @ (beta*A)^T + (alpha*c)^3 + beta*s
    out = x + D' - gp * sum_h(D'_h)

Pipeline per [128, 4096] row tile:
  PE   : 32 fp32 transposes (d onto partitions) + 32 head matmuls with a
         257th column in (beta*A)^T valued -alpha/DH, so PSUM col 256 is
         -alpha*mean directly.
  DVE  : one custom fused op per head segment computes
         (alpha*x - alpha*m)^3 + lin reading lin AND the mean column
         straight from PSUM (no ACT copy-out of lin at all).
  drift = beta*s + cl via scalar_tensor_tensor split DVE/GpSimd;
  head-sum tree + broadcast mean-coupling + final x+dd split DVE/GpSimd.

A-setup is 3 batched DMAs; diag is embedded via an extra accumulating
matmul against a diagonal matrix built with one affine_select.

Sharding: batch B=8192 split across 8 cores (1024 rows each), params
replicated.
"""

import sys

for _p in ("/opt/trn_rl_repo",):
    if _p not in sys.path:
        sys.path.insert(0, _p)

import math
import re
from contextlib import ExitStack

import numpy as np

import concourse.bass as bass
import concourse.tile as tile
from concourse import bacc, mybir
from concourse.bass import broadcast_tensor_aps
from concourse.bass_utils import run_bass_kernel_spmd
from concourse.masks import make_identity

F32 = mybir.dt.float32
BF16 = mybir.dt.bfloat16
MID = mybir.dt.float16
AOP = mybir.AluOpType

B = 8192
D = 4096
H = 16
DH = 256
R = 64
DT = 0.05
NCORES = 8
BS = B // NCORES          # rows per core = 1024
P = 128
NT = BS // P              # row tiles per core = 8
NCH = D // P              # 128-wide column chunks per row tile = 32

# Column splits of the two full-width passes shared between DVE and GpSimd.
STT_DVE_COLS = 1024       # drift = beta*s + cl
FIN_DVE_COLS = 1024       # out = x + dd


# ---- custom DVE op: out = (in0*s0 + s1)^3 + in1 -------------------------
def _register_cubic_op():
    from concourse import dve_ops
    from concourse.dve_spec import C0, C1, Spec, Src0, Src1, sq
    from concourse.dve_table_gen import dve_ver_for

    name = "CUBIC_LIN_ANT"
    if name in dve_ops._SUB_OPCODE_FOR_NAME:
        return next(op for op in dve_ops.OPS if op.name == name)

    y = Src0 * C0 + C1
    spec = Spec(
        body=sq(y) * y + Src1,
        reference=lambda in0, in1, s0, s1, imm2: (
            (in0.astype(np.float32) * s0 + s1) ** 2
            * (in0.astype(np.float32) * s0 + s1)
            + in1
        ).astype(np.float32),
    )
    op = dve_ops.DveOp(name, spec, subdim=False, uops_sha={})
    dve_ops.OPS.append(op)
    dve_ops.CUSTOM_DVE_SPECS[name] = spec
    dve_ops._SUB_OPCODE_FOR_NAME[name] = (
        max(dve_ops._SUB_OPCODE_FOR_NAME.values()) + 1
    )
    ver = dve_ver_for("TRN2")
    try:
        op.compile(ver)
    except ValueError as e:
        m = re.search(rf"{ver}: ([0-9a-f]+)", str(e))
        op.uops_sha[ver] = m.group(1)
        op.compile(ver)
    return op


CUBIC_OP = _register_cubic_op()


def _emit(tc: tile.TileContext, aps: dict, cubic_scale: float, coupling: float):
    nc = tc.nc
    beta = DT * (1.0 + coupling)
    gp = DT * coupling / (H * beta)
    alpha = (beta * cubic_scale) ** (1.0 / 3.0)

    state = aps["state"]
    signal = aps["signal"]
    U_d = aps["U"]
    V_d = aps["V"]
    diag_d = aps["diag"]
    out_d = aps["out"]

    with ExitStack() as ctx:
        consts = ctx.enter_context(tc.tile_pool(name="consts", bufs=1))

        ident = consts.tile([P, P], F32, tag="ident")
        make_identity(nc, ident)
        ident_bf = consts.tile([P, P], BF16, tag="ident_bf")
        make_identity(nc, ident_bf)

        # (beta*A)^T with the mean column: [d-part, head, chunk, 257].
        # Col 256 = -alpha/DH so PSUM col 256 accumulates -alpha*mean.
        AT = consts.tile([P, H, 2, DH + 1], BF16, tag="AT")
        nc.gpsimd.memset(AT[:, :, :, DH:DH + 1], -alpha / DH)

        # --- one-time A setup (batched DMAs, diag via matmul) ---
        with (
            tc.tile_pool(name="setup", bufs=1) as setup,
            tc.tile_pool(name="setup2", bufs=2) as setup2,
            tc.tile_pool(name="setup_ps", bufs=2, space="PSUM") as setup_ps,
            tc.tile_pool(name="setup_ps2", bufs=3, space="PSUM") as setup_ps2,
        ):
            u_s = setup.tile([P, H, 2, R], F32, tag="u_s")
            nc.sync.dma_start(
                out=u_s, in_=U_d.rearrange("h (k p) r -> p h k r", p=P)
            )
            v_s = setup.tile([R, H, DH], F32, tag="v_s")
            nc.scalar.dma_start(out=v_s, in_=V_d.rearrange("h r e -> r h e"))
            d_hs = setup.tile([H, DH], F32, tag="d_hs")
            nc.sync.dma_start(out=d_hs, in_=diag_d)

            u_b = setup.tile([P, H, 2, R], BF16, tag="u_b")
            nc.vector.tensor_copy(u_b, u_s)
            v_b = setup.tile([R, H, DH], BF16, tag="v_b")
            nc.vector.tensor_copy(v_b, v_s)

            # diag values onto partitions: dcol[p, k, h] = d[h, k*128+p]
            dcol_ps = setup_ps.tile([P, 2, H], F32, tag="dcol_ps")
            for k in range(2):
                nc.tensor.transpose(
                    dcol_ps[:, k, :], d_hs[:, k * P:(k + 1) * P], ident[0:H, 0:H]
                )
            dcol = setup.tile([P, H, 2], F32, tag="dcol")
            nc.scalar.copy(out=dcol.rearrange("p h k -> p k h"), in_=dcol_ps)

            # dmask[k] = 1 at (p, e=k*128+p); diagall[p,h,k,q] = d at q==p
            dmask = setup.tile([P, 2, DH], BF16, tag="dmask")
            nc.gpsimd.memset(dmask, 0.0)
            for k in range(2):
                nc.gpsimd.affine_select(
                    out=dmask[:, k, :], in_=dmask[:, k, :],
                    compare_op=AOP.not_equal, fill=1.0,
                    base=-(k * P), pattern=[[1, DH]], channel_multiplier=-1,
                )
            diagall = setup.tile([P, H * 2, P], BF16, tag="diagall")
            da_in = dcol.rearrange("p h k -> p (h k)")[:, :, None]
            da_out_b, da_in_b = broadcast_tensor_aps(diagall[:, :, :], da_in)
            nc.gpsimd.affine_select(
                out=da_out_b, in_=da_in_b,
                compare_op=AOP.is_equal, fill=0.0,
                base=0, pattern=[[0, H * 2], [1, P]], channel_multiplier=-1,
            )

            # U_h^T via PE transpose (bf16): ut[r, h, k, :]
            ut_b = setup.tile([R, H, 2, P], BF16, tag="ut_b")
            for g in range(4):
                ut_ps = setup_ps.tile([R, H // 4, 2, P], BF16, tag="ut_ps")
                for hh in range(H // 4):
                    h = g * (H // 4) + hh
                    for k in range(2):
                        nc.tensor.transpose(
                            ut_ps[:, hh, k, :], u_b[:, h, k, :], ident_bf
                        )
                nc.scalar.copy(
                    out=ut_b[:, g * (H // 4):(g + 1) * (H // 4), :, :], in_=ut_ps
                )

            # A chunks: V^T U^T plus diag embedded via second matmul
            for h in range(H):
                a_ps = setup_ps2.tile([P, 2, DH], F32, tag="a_ps")
                for k in range(2):
                    nc.tensor.matmul(
                        a_ps[:, k, :],
                        lhsT=v_b[:, h, k * P:(k + 1) * P],
                        rhs=ut_b[:, h, :, :].rearrange("r a b -> r (a b)"),
                        start=True, stop=False,
                    )
                    nc.tensor.matmul(
                        a_ps[:, k, :],
                        lhsT=diagall[:, h * 2 + k, :],
                        rhs=dmask[:, k, :],
                        start=False, stop=True,
                    )
                nc.scalar.mul(AT[:, h, :, 0:DH], a_ps, beta)

        # --- main loop pools ---
        xp = ctx.enter_context(tc.tile_pool(name="xp", bufs=3))
        sp = ctx.enter_context(tc.tile_pool(name="sp", bufs=2))
        sbp = ctx.enter_context(tc.tile_pool(name="sbp", bufs=2))
        hp = ctx.enter_context(tc.tile_pool(name="hp", bufs=2))
        clp = ctx.enter_context(tc.tile_pool(name="clp", bufs=2))
        drp = ctx.enter_context(tc.tile_pool(name="drp", bufs=2))
        trp = ctx.enter_context(tc.tile_pool(name="trp", bufs=2))
        ps_tp = ctx.enter_context(tc.tile_pool(name="ps_tp", bufs=2, space="PSUM"))
        ps_lin = ctx.enter_context(tc.tile_pool(name="ps_lin", bufs=3, space="PSUM"))

        for it in range(NT):
            r0 = it * P
            x_t = xp.tile([P, D], F32, tag="x", name="x_t")
            nc.sync.dma_start(out=x_t, in_=state[r0:r0 + P, :])
            s_t = sp.tile([P, D], F32, tag="s", name="s_t")
            nc.scalar.dma_start(out=s_t, in_=signal[r0:r0 + P, :])

            # Transpose the 32 fp32 chunks of x (d onto partitions), cast
            # to bf16 in the PSUM->SBUF copy.
            hT = hp.tile([P, NCH, P], BF16, tag="hT", name="hT")
            for g in range(8):
                tp_ps = ps_tp.tile([P, 4 * P], F32, tag="tp_ps", name="tp_ps")
                for c in range(4):
                    j = g * 4 + c
                    nc.tensor.transpose(
                        tp_ps[:, c * P:(c + 1) * P],
                        x_t[:, j * P:(j + 1) * P], ident,
                    )
                nc.scalar.copy(
                    out=hT[:, g * 4:(g + 1) * 4, :].rearrange("p a b -> p (a b)"),
                    in_=tp_ps,
                )

            # Head matmuls (257 cols: lin | -alpha*mean), then the fused
            # cubic: cl = (alpha*x - alpha*m)^3 + lin, lin/mean from PSUM.
            cl_t = clp.tile([P, D], MID, tag="cl", name="cl_t")
            for hp2 in range(H // 2):
                l_ps = ps_lin.tile([P, 2, 512], F32, tag="l_ps", name="l_ps")
                for hh in range(2):
                    h = hp2 * 2 + hh
                    for k in range(2):
                        nc.tensor.matmul(
                            l_ps[:, hh, 0:DH + 1],
                            lhsT=hT[:, 2 * h + k, :], rhs=AT[:, h, k, :],
                            start=(k == 0), stop=(k == 1),
                        )
                for hh in range(2):
                    h = hp2 * 2 + hh
                    nc.vector._custom_dve(
                        CUBIC_OP,
                        out=cl_t[:, h * DH:(h + 1) * DH],
                        in0=x_t[:, h * DH:(h + 1) * DH],
                        in1=l_ps[:, hh, 0:DH],
                        s0=alpha,
                        s1=l_ps[:, hh, DH:DH + 1],
                    )

            # sb = beta*s on ACT (has slack); drift = sb + cl on DVE (fp16 2x)
            sb_t = sbp.tile([P, D], MID, tag="sb", name="sb_t")
            nc.scalar.mul(sb_t, s_t, beta)
            dr_t = drp.tile([P, D], MID, tag="dr", name="dr_t")
            nc.vector.tensor_add(dr_t, sb_t, cl_t)

            # head-sum tree -> mhn = -gp * sum_h(drift_h)
            # first (largest) level on GpSimd, rest on DVE
            t8 = trp.tile([P, D // 2], MID, tag="t8", name="t8")
            nc.gpsimd.tensor_add(t8, dr_t[:, 0:D // 2], dr_t[:, D // 2:D])
            t4 = trp.tile([P, D // 4], MID, tag="t4", name="t4")
            nc.vector.tensor_add(t4, t8[:, 0:D // 4], t8[:, D // 4:D // 2])
            t2r = trp.tile([P, D // 8], MID, tag="t2r", name="t2r")
            nc.vector.tensor_add(t2r, t4[:, 0:D // 8], t4[:, D // 8:D // 4])
            mhn = trp.tile([P, DH], MID, tag="mhn", name="mhn")
            nc.vector.tensor_add(mhn, t2r[:, 0:DH], t2r[:, DH:2 * DH])
            nc.vector.tensor_scalar_mul(mhn, mhn, -gp)

            # dd = drift + mhn (broadcast over the 16 heads); reuse cl buf
            dd_t = cl_t
            dd_v = dd_t.rearrange("p (h e) -> p h e", h=H)
            dr_v = dr_t.rearrange("p (h e) -> p h e", h=H)
            mh_v = mhn[:, None, :]
            dr_b, mh_b = broadcast_tensor_aps(dr_v, mh_v)
            nc.vector.tensor_tensor(
                out=dd_v, in0=dr_b, in1=mh_b, op=AOP.add
            )

            # out = x + dd (split DVE / GpSimd; into x buffer)
            o_t = x_t
            fd = FIN_DVE_COLS
            nc.vector.tensor_add(o_t[:, 0:fd], x_t[:, 0:fd], dd_t[:, 0:fd])
            nc.gpsimd.tensor_add(o_t[:, fd:D], x_t[:, fd:D], dd_t[:, fd:D])
            if it % 2 == 0:
                nc.sync.dma_start(out=out_d[r0:r0 + P, :], in_=o_t)
            else:
                nc.scalar.dma_start(out=out_d[r0:r0 + P, :], in_=o_t)


_CACHE: dict = {}


def _build(cubic_scale: float, coupling: float) -> bass.Bass:
    key = (float(cubic_scale), float(coupling), STT_DVE_COLS, FIN_DVE_COLS)
    if key in _CACHE:
        return _CACHE[key]
    nc = bacc.Bacc("TRN2", target_bir_lowering=False, debug=False)
    aps = {
        "state": nc.dram_tensor("state", [BS, D], F32, kind="ExternalInput").ap(),
        "signal": nc.dram_tensor("signal", [BS, D], F32, kind="ExternalInput").ap(),
        "U": nc.dram_tensor("U", [H, DH, R], F32, kind="ExternalInput").ap(),
        "V": nc.dram_tensor("V", [H, R, DH], F32, kind="ExternalInput").ap(),
        "diag": nc.dram_tensor("diag", [H, DH], F32, kind="ExternalInput").ap(),
        "out": nc.dram_tensor("out", [BS, D], F32, kind="ExternalOutput").ap(),
    }
    with tile.TileContext(nc) as tc:
        _emit(tc, aps, float(cubic_scale), float(coupling))
    nc.compile()
    _CACHE[key] = nc
    return nc


def run(state, signal, U, V, diag, cubic_scale, coupling, trace=False):
    state = np.ascontiguousarray(np.asarray(state, dtype=np.float32))
    signal = np.ascontiguousarray(np.asarray(signal, dtype=np.float32))
    U = np.ascontiguousarray(np.asarray(U, dtype=np.float32))
    V = np.ascontiguousarray(np.asarray(V, dtype=np.float32))
    diag = np.ascontiguousarray(np.asarray(diag, dtype=np.float32))

    nc = _build(float(cubic_scale), float(coupling))
    in_maps = []
    for i in range(NCORES):
        sl = slice(i * BS, (i + 1) * BS)
        in_maps.append({
            "state": state[sl], "signal": signal[sl],
            "U": U, "V": V, "diag": diag,
        })
    res = run_bass_kernel_spmd(nc, in_maps, list(range(NCORES)), trace=trace)
    out = np.concatenate([res.results[i]["out"] for i in range(NCORES)], axis=0)
    return out, res


def kernel(state, signal, U, V, diag, cubic_scale, coupling) -> np.ndarray:
    out, _ = run(state, signal, U, V, diag, cubic_scale, coupling, trace=False)
    return out


# revision 11
# speedup vs baseline: 1.3597x; 1.0600x over previous
"""Trainium2 Bass kernel for nn_MultiHeadDynamics.

Computation (per sample row x of state, s of signal):
    heads   = x.reshape(H, DH)                      # H=16, DH=256
    A_h     = U_h @ V_h + diag(d_h)                 # (DH, DH) per head
    lin     = heads @ A_h^T
    c       = heads - mean_dh(heads)
    drift   = lin + cs * c^3 + s
    out     = x + DT*(1+cp)*drift - (DT*cp/H) * sum_h(drift_h)

Folding:  beta = DT*(1+cp);  gp = DT*cp/(H*beta);  alpha = cbrt(beta*cs)
    D'  = beta*drift = heads @ (beta*A)^T + (alpha*c)^3 + beta*s
    out = x + D' - gp * sum_h(D'_h)

Pipeline per [128, 4096] row tile:
  PE   : 32 fp32 transposes (d onto partitions); per head pair 4 matmuls
         against (beta*A)^T widened with a 257th column valued -alpha/DH
         (so PSUM col 256 accumulates -alpha*mean), plus 2 identity
         matmuls accumulating beta*s (bf16, from ACT) into the lin PSUM.
  DVE  : one custom fused op per head computes the complete drift
         (alpha*x - alpha*m)^3 + (lin + beta*s), reading lin AND the
         mean column straight from PSUM. Tail: head-sum tree (first
         level split with GpSimd), broadcast mean-coupling add, and the
         final fp32 x+dd split DVE/GpSimd.

A-setup: 3 batched DMAs on the scalar queue; beta folded into the U
cast; diag embedded via an extra accumulating matmul against a diagonal
matrix built with one affine_select; per-head AT tiles so the first
row-tile's matmuls start as soon as head 0 is ready.

Sharding: batch B=8192 split across 8 cores (1024 rows each), params
replicated.
"""

import sys

for _p in ("/opt/trn_rl_repo",):
    if _p not in sys.path:
        sys.path.insert(0, _p)

import math
import re
from contextlib import ExitStack

import numpy as np

import concourse.bass as bass
import concourse.tile as tile
from concourse import bacc, mybir
from concourse.bass import broadcast_tensor_aps
from concourse.bass_utils import run_bass_kernel_spmd
from concourse.masks import make_identity

F32 = mybir.dt.float32
BF16 = mybir.dt.bfloat16
MID = mybir.dt.float16
AOP = mybir.AluOpType

B = 8192
D = 4096
H = 16
DH = 256
R = 64
DT = 0.05
NCORES = 8
BS = B // NCORES          # rows per core = 1024
P = 128
NT = BS // P              # row tiles per core = 8
NCH = D // P              # 128-wide column chunks per row tile = 32

# Column splits of work shared between DVE and GpSimd (tuning knobs).
T8_GP_COLS = 1024         # of the 2048-col first tree level, GpSimd share
FIN_DVE_COLS = 1024       # of the 4096-col final x+dd, DVE share


# ---- custom DVE op: out = (in0*s0 + s1)^3 + in1 -------------------------
def _register_cubic_op():
    from concourse import dve_ops
    from concourse.dve_spec import C0, C1, Spec, Src0, Src1, sq
    from concourse.dve_table_gen import dve_ver_for

    name = "CUBIC_LIN_ANT"
    if name in dve_ops._SUB_OPCODE_FOR_NAME:
        return next(op for op in dve_ops.OPS if op.name == name)

    y = Src0 * C0 + C1
    spec = Spec(
        body=sq(y) * y + Src1,
        reference=lambda in0, in1, s0, s1, imm2: (
            (in0.astype(np.float32) * s0 + s1) ** 2
            * (in0.astype(np.float32) * s0 + s1)
            + in1
        ).astype(np.float32),
    )
    op = dve_ops.DveOp(name, spec, subdim=False, uops_sha={})
    dve_ops.OPS.append(op)
    dve_ops.CUSTOM_DVE_SPECS[name] = spec
    dve_ops._SUB_OPCODE_FOR_NAME[name] = (
        max(dve_ops._SUB_OPCODE_FOR_NAME.values()) + 1
    )
    ver = dve_ver_for("TRN2")
    try:
        op.compile(ver)
    except ValueError as e:
        m = re.search(rf"{ver}: ([0-9a-f]+)", str(e))
        op.uops_sha[ver] = m.group(1)
        op.compile(ver)
    return op


CUBIC_OP = _register_cubic_op()


def _emit(tc: tile.TileContext, aps: dict, cubic_scale: float, coupling: float):
    nc = tc.nc
    beta = DT * (1.0 + coupling)
    gp = DT * coupling / (H * beta)
    alpha = (beta * cubic_scale) ** (1.0 / 3.0)

    state = aps["state"]
    signal = aps["signal"]
    U_d = aps["U"]
    V_d = aps["V"]
    diag_d = aps["diag"]
    out_d = aps["out"]

    with ExitStack() as ctx:
        consts = ctx.enter_context(tc.tile_pool(name="consts", bufs=1))

        ident = consts.tile([P, P], F32, tag="ident")
        make_identity(nc, ident)
        ident_bf = consts.tile([P, P], BF16, tag="ident_bf")
        make_identity(nc, ident_bf)

        # Per-head (beta*A)^T with the mean column: [d-part, chunk, 257].
        # Col 256 = -alpha/DH so PSUM col 256 accumulates -alpha*mean.
        ATs = [
            consts.tile([P, 2, DH + 1], BF16, tag=f"AT{h}", name=f"AT{h}")
            for h in range(H)
        ]

        # --- one-time A setup (batched DMAs on the scalar queue) ---
        with (
            tc.tile_pool(name="setup", bufs=1) as setup,
            tc.tile_pool(name="setup_ps", bufs=2, space="PSUM") as setup_ps,
            tc.tile_pool(name="setup_ps2", bufs=3, space="PSUM") as setup_ps2,
        ):
            u_s = setup.tile([P, H, 2, R], F32, tag="u_s")
            nc.scalar.dma_start(
                out=u_s, in_=U_d.rearrange("h (k p) r -> p h k r", p=P)
            )
            v_s = setup.tile([R, H, DH], F32, tag="v_s")
            nc.scalar.dma_start(out=v_s, in_=V_d.rearrange("h r e -> r h e"))
            d_hs = setup.tile([H, DH], F32, tag="d_hs")
            nc.scalar.dma_start(out=d_hs, in_=diag_d)

            # beta folded into the U cast so AT copies need no scaling
            u_b = setup.tile([P, H * 2 * R], BF16, tag="u_b")
            nc.vector.tensor_scalar(
                out=u_b, in0=u_s.rearrange("p h k r -> p (h k r)"),
                scalar1=beta, scalar2=None, op0=AOP.mult,
            )
            u_bv = u_b.rearrange("p (h k r) -> p h k r", h=H, k=2)
            v_b = setup.tile([R, H * DH], BF16, tag="v_b")
            nc.vector.tensor_copy(
                v_b, v_s.rearrange("r h e -> r (h e)")
            )
            v_bv = v_b.rearrange("r (h e) -> r h e", h=H)

            # diag values onto partitions, beta-scaled:
            # dcol[p, h, k] = beta * d[h, k*128+p]
            dcol_ps = setup_ps.tile([P, 2, H], F32, tag="dcol_ps")
            for k in range(2):
                nc.tensor.transpose(
                    dcol_ps[:, k, :], d_hs[:, k * P:(k + 1) * P], ident[0:H, 0:H]
                )
            dcol = setup.tile([P, H, 2], F32, tag="dcol")
            nc.scalar.mul(dcol.rearrange("p h k -> p k h"), dcol_ps, beta)

            # dmask[k] = 1 at (p, e=k*128+p); diagall[p, h*2+k, q] = dcol at q==p
            dmask = setup.tile([P, 2, DH], BF16, tag="dmask")
            nc.gpsimd.memset(dmask, 0.0)
            for k in range(2):
                nc.gpsimd.affine_select(
                    out=dmask[:, k, :], in_=dmask[:, k, :],
                    compare_op=AOP.not_equal, fill=1.0,
                    base=-(k * P), pattern=[[1, DH]], channel_multiplier=-1,
                )
            diagall = setup.tile([P, H * 2, P], BF16, tag="diagall")
            da_in = dcol.rearrange("p h k -> p (h k)")[:, :, None]
            da_out_b, da_in_b = broadcast_tensor_aps(diagall[:, :, :], da_in)
            nc.gpsimd.affine_select(
                out=da_out_b, in_=da_in_b,
                compare_op=AOP.is_equal, fill=0.0,
                base=0, pattern=[[0, H * 2], [1, P]], channel_multiplier=-1,
            )

            # U_h^T via PE transpose (bf16): ut[r, h, k, :]
            ut_b = setup.tile([R, H, 2, P], BF16, tag="ut_b")
            for g in range(4):
                ut_ps = setup_ps.tile([R, H // 4, 2, P], BF16, tag="ut_ps")
                for hh in range(H // 4):
                    h = g * (H // 4) + hh
                    for k in range(2):
                        nc.tensor.transpose(
                            ut_ps[:, hh, k, :], u_bv[:, h, k, :], ident_bf
                        )
                nc.scalar.copy(
                    out=ut_b[:, g * (H // 4):(g + 1) * (H // 4), :, :], in_=ut_ps
                )

            # A chunks: beta*(V^T U^T) plus beta*diag via second matmul;
            # copy-out alternates ACT / DVE. Mean column via tiny memsets.
            for h in range(H):
                a_ps = setup_ps2.tile([P, 2, DH], F32, tag="a_ps")
                for k in range(2):
                    nc.tensor.matmul(
                        a_ps[:, k, :],
                        lhsT=v_bv[:, h, k * P:(k + 1) * P],
                        rhs=ut_b[:, h, :, :].rearrange("r a b -> r (a b)"),
                        start=True, stop=False,
                    )
                    nc.tensor.matmul(
                        a_ps[:, k, :],
                        lhsT=diagall[:, h * 2 + k, :],
                        rhs=dmask[:, k, :],
                        start=False, stop=True,
                    )
                nc.gpsimd.memset(ATs[h][:, :, DH:DH + 1], -alpha / DH)
                if h % 2 == 0:
                    nc.scalar.copy(out=ATs[h][:, :, 0:DH], in_=a_ps)
                else:
                    nc.vector.tensor_copy(ATs[h][:, :, 0:DH], a_ps)

        # --- main loop pools ---
        xp = ctx.enter_context(tc.tile_pool(name="xp", bufs=3))
        sp = ctx.enter_context(tc.tile_pool(name="sp", bufs=2))
        sbp = ctx.enter_context(tc.tile_pool(name="sbp", bufs=2))
        hp = ctx.enter_context(tc.tile_pool(name="hp", bufs=2))
        clp = ctx.enter_context(tc.tile_pool(name="clp", bufs=2))
        ddp = ctx.enter_context(tc.tile_pool(name="ddp", bufs=2))
        trp = ctx.enter_context(tc.tile_pool(name="trp", bufs=2))
        ps_tp = ctx.enter_context(tc.tile_pool(name="ps_tp", bufs=2, space="PSUM"))
        ps_lin = ctx.enter_context(tc.tile_pool(name="ps_lin", bufs=3, space="PSUM"))

        for it in range(NT):
            r0 = it * P
            x_t = xp.tile([P, D], F32, tag="x", name="x_t")
            nc.sync.dma_start(out=x_t, in_=state[r0:r0 + P, :])
            s_t = sp.tile([P, D], F32, tag="s", name="s_t")
            nc.scalar.dma_start(out=s_t, in_=signal[r0:r0 + P, :])

            # beta*s in bf16 (ACT), consumed by the identity matmuls
            sb_t = sbp.tile([P, D], BF16, tag="sb", name="sb_t")
            nc.scalar.mul(sb_t, s_t, beta)

            # Transpose the 32 fp32 chunks of x (d onto partitions), cast
            # to bf16 in the PSUM->SBUF copy.
            hT = hp.tile([P, NCH, P], BF16, tag="hT", name="hT")
            for g in range(8):
                tp_ps = ps_tp.tile([P, 4 * P], F32, tag="tp_ps", name="tp_ps")
                for c in range(4):
                    j = g * 4 + c
                    nc.tensor.transpose(
                        tp_ps[:, c * P:(c + 1) * P],
                        x_t[:, j * P:(j + 1) * P], ident,
                    )
                nc.scalar.copy(
                    out=hT[:, g * 4:(g + 1) * 4, :].rearrange("p a b -> p (a b)"),
                    in_=tp_ps,
                )

            # Per head pair: 4 matmuls (257 cols: lin | -alpha*mean), 2
            # identity matmuls folding beta*s into lin, then the fused
            # cubic producing the full drift from PSUM.
            cl_t = clp.tile([P, D], MID, tag="cl", name="cl_t")
            for hp2 in range(H // 2):
                l_ps = ps_lin.tile([P, 2, 512], F32, tag="l_ps", name="l_ps")
                for hh in range(2):
                    h = hp2 * 2 + hh
                    for k in range(2):
                        nc.tensor.matmul(
                            l_ps[:, hh, 0:DH + 1],
                            lhsT=hT[:, 2 * h + k, :], rhs=ATs[h][:, k, :],
                            start=(k == 0), stop=False,
                        )
                for hh in range(2):
                    h = hp2 * 2 + hh
                    nc.tensor.matmul(
                        l_ps[:, hh, 0:DH],
                        lhsT=ident_bf, rhs=sb_t[:, h * DH:(h + 1) * DH],
                        start=False, stop=True,
                    )
                for hh in range(2):
                    h = hp2 * 2 + hh
                    nc.vector._custom_dve(
                        CUBIC_OP,
                        out=cl_t[:, h * DH:(h + 1) * DH],
                        in0=x_t[:, h * DH:(h + 1) * DH],
                        in1=l_ps[:, hh, 0:DH],
                        s0=alpha,
                        s1=l_ps[:, hh, DH:DH + 1],
                    )

            # head-sum tree -> mhn = -gp * sum_h(drift_h)
            # first (largest) level split GpSimd / DVE
            t8 = trp.tile([P, D // 2], MID, tag="t8", name="t8")
            tg = T8_GP_COLS
            nc.gpsimd.tensor_add(
                t8[:, 0:tg], cl_t[:, 0:tg], cl_t[:, D // 2:D // 2 + tg]
            )
            nc.vector.tensor_add(
                t8[:, tg:D // 2], cl_t[:, tg:D // 2], cl_t[:, D // 2 + tg:D]
            )
            t4 = trp.tile([P, D // 4], MID, tag="t4", name="t4")
            nc.vector.tensor_add(t4, t8[:, 0:D // 4], t8[:, D // 4:D // 2])
            t2r = trp.tile([P, D // 8], MID, tag="t2r", name="t2r")
            nc.vector.tensor_add(t2r, t4[:, 0:D // 8], t4[:, D // 8:D // 4])
            mhn = trp.tile([P, DH], MID, tag="mhn", name="mhn")
            nc.vector.tensor_add(mhn, t2r[:, 0:DH], t2r[:, DH:2 * DH])
            nc.vector.tensor_scalar_mul(mhn, mhn, -gp)

            # dd = drift + mhn (broadcast over the 16 heads)
            dd_t = ddp.tile([P, D], MID, tag="dd", name="dd_t")
            dd_v = dd_t.rearrange("p (h e) -> p h e", h=H)
            cl_v = cl_t.rearrange("p (h e) -> p h e", h=H)
            mh_v = mhn[:, None, :]
            cl_b, mh_b = broadcast_tensor_aps(cl_v, mh_v)
            nc.vector.tensor_tensor(out=dd_v, in0=cl_b, in1=mh_b, op=AOP.add)

            # out = x + dd (split DVE / GpSimd; into x buffer)
            o_t = x_t
            fd = FIN_DVE_COLS
            nc.vector.tensor_add(o_t[:, 0:fd], x_t[:, 0:fd], dd_t[:, 0:fd])
            nc.gpsimd.tensor_add(o_t[:, fd:D], x_t[:, fd:D], dd_t[:, fd:D])
            if it == NT - 1:
                nc.sync.dma_start(
                    out=out_d[r0:r0 + P, 0:D // 2], in_=o_t[:, 0:D // 2]
                )
                nc.scalar.dma_start(
                    out=out_d[r0:r0 + P, D // 2:D], in_=o_t[:, D // 2:D]
                )
            elif it % 2 == 0:
                nc.sync.dma_start(out=out_d[r0:r0 + P, :], in_=o_t)
            else:
                nc.scalar.dma_start(out=out_d[r0:r0 + P, :], in_=o_t)


_CACHE: dict = {}


def _build(cubic_scale: float, coupling: float) -> bass.Bass:
    key = (float(cubic_scale), float(coupling), T8_GP_COLS, FIN_DVE_COLS)
    if key in _CACHE:
        return _CACHE[key]
    nc = bacc.Bacc("TRN2", target_bir_lowering=False, debug=False)
    aps = {
        "state": nc.dram_tensor("state", [BS, D], F32, kind="ExternalInput").ap(),
        "signal": nc.dram_tensor("signal", [BS, D], F32, kind="ExternalInput").ap(),
        "U": nc.dram_tensor("U", [H, DH, R], F32, kind="ExternalInput").ap(),
        "V": nc.dram_tensor("V", [H, R, DH], F32, kind="ExternalInput").ap(),
        "diag": nc.dram_tensor("diag", [H, DH], F32, kind="ExternalInput").ap(),
        "out": nc.dram_tensor("out", [BS, D], F32, kind="ExternalOutput").ap(),
    }
    with tile.TileContext(nc) as tc:
        _emit(tc, aps, float(cubic_scale), float(coupling))
    nc.compile()
    _CACHE[key] = nc
    return nc


def run(state, signal, U, V, diag, cubic_scale, coupling, trace=False):
    state = np.ascontiguousarray(np.asarray(state, dtype=np.float32))
    signal = np.ascontiguousarray(np.asarray(signal, dtype=np.float32))
    U = np.ascontiguousarray(np.asarray(U, dtype=np.float32))
    V = np.ascontiguousarray(np.asarray(V, dtype=np.float32))
    diag = np.ascontiguousarray(np.asarray(diag, dtype=np.float32))

    nc = _build(float(cubic_scale), float(coupling))
    in_maps = []
    for i in range(NCORES):
        sl = slice(i * BS, (i + 1) * BS)
        in_maps.append({
            "state": state[sl], "signal": signal[sl],
            "U": U, "V": V, "diag": diag,
        })
    res = run_bass_kernel_spmd(nc, in_maps, list(range(NCORES)), trace=trace)
    out = np.concatenate([res.results[i]["out"] for i in range(NCORES)], axis=0)
    return out, res


def kernel(state, signal, U, V, diag, cubic_scale, coupling) -> np.ndarray:
    out, _ = run(state, signal, U, V, diag, cubic_scale, coupling, trace=False)
    return out
